# revision 21
# baseline (speedup 1.0000x reference)
"""Mixture-of-Softmaxes with shared embedding — 8-core Trainium2 Bass kernel.

Strategy (tensor-parallel on the vocab output head, per the sharding hint):
  - Vocab dim V is sharded across the 8 cores (Vp = 6283 rows each, zero-padded
    from 50257 to 50264; the 7 pad rows contribute exactly exp(0)=1 to each
    softmax denominator and are corrected by a constant subtraction).
  - The expert transforms (10 experts x 2560x2560) are sharded as 200
    (expert, d-block) jobs, 25 per core, followed by an AllGather of the
    fp8 expert_hidden^T (6.5 MB, split 17/8 so the first piece overlaps the
    tail of the expert matmuls).
  - The big vocab matmul runs in fp8 (e4m3) with DoubleRow perf mode
    (K=256 per instruction): embedding is scaled x64 on the host,
    expert_hidden x16 on chip, undone by the fused exp scale (1/1024).
  - Softmax over the full vocab needs a cross-core reduction: since the
    logits are O(1), exp() cannot overflow in fp32, so the max-shift is
    unnecessary and the reduction collapses to one AllReduce-ADD of the
    (128,10) sum-of-exp stats per token-half.
  - RMSNorm is folded in linearly: norm_scale is folded into the expert and
    gate weights on the host; the per-token 1/rms factor is computed on-chip
    in fp32 and applied during the PSUM->SBUF copy of expert_hidden
    (free-dim broadcast), so the expert matmuls never wait on it.
  - All DRAM layouts are partition-major so DMA descriptors move >=5KB
    contiguous runs per partition (descriptor-rate, not bytes, was the
    limiting factor for the spill/reload streams).

kernel(**inputs) takes the full unsharded inputs and returns the full
(1, 256, 50257) float32 logits.
"""
import os
import sys

for _p in ("/opt/trn_rl_repo",):
    if _p not in sys.path:
        sys.path.append(_p)

import numpy as np
import ml_dtypes

import concourse.bass as bass
import concourse.bacc as bacc
import concourse.mybir as mybir
import concourse.tile as tile
from concourse.bass_utils import run_bass_kernel_spmd
from concourse.masks import make_identity

BF16 = ml_dtypes.bfloat16
FP8 = ml_dtypes.float8_e4m3

NCORES = 8
S = 256          # tokens (B*S)
H = 2560         # hidden
E = 10           # experts
V = 50257        # vocab
KB = H // 128    # 20 k-blocks
NJOBS = E * KB   # 200 (expert, d-block) jobs
JPC = NJOBS // NCORES  # 25 jobs per core
AGP = (9, 8, 8)  # jobs per core in each AllGather piece
AGOFF = (0, 9, 17)  # slot offsets of the pieces
VP = 6283        # per-core vocab slice (8*6283 = 50264)
NPAD = NCORES * VP - V  # 7 zero-pad vocab rows (on the last core)
CHUNK = 512
NCH = (VP + CHUNK - 1) // CHUNK  # 13
CHUNKS = [(i * CHUNK, min(CHUNK, VP - i * CHUNK)) for i in range(NCH)]
EPS_NORM = 1e-05
EPS_LOG = 1e-10
EMB_SCALE = 64.0
EH_SCALE = 16.0
INV_SCALE = 1.0 / (EMB_SCALE * EH_SCALE)
GP_E = (7, 8, 9)   # experts mixed on GpSimd in pass 2 (rest on DVE)

_nc_cache = None


def _job_of(c, slot):
    """Global job id (= e*KB + k) held by core c at slot (0..JPC-1).

    AG piece p carries a rank-major contiguous range of global jobs, so
    low experts are complete as soon as the early pieces land.
    """
    for p, (n, off) in enumerate(zip(AGP, AGOFF)):
        if slot < off + n:
            base = NCORES * sum(AGP[:p])
            return base + c * n + (slot - off)
    raise ValueError(slot)


def build_kernel():
    global _nc_cache
    if _nc_cache is not None:
        return _nc_cache
    f32 = mybir.dt.float32
    bf = mybir.dt.bfloat16
    f8 = mybir.dt.float8e4
    u8 = mybir.dt.uint8
    nc = bacc.Bacc("TRN2", target_bir_lowering=False, debug=False, num_devices=NCORES)

    h32 = nc.declare_dram_parameter("h32", [2, 128, H], f32, isOutput=False)
    hT = nc.declare_dram_parameter("hT", [128, KB, S], bf, isOutput=False)
    gw = nc.declare_dram_parameter("gw", [KB, 128, E], bf, isOutput=False)
    wj = nc.declare_dram_parameter("wjobs", [JPC, 128, KB, 128], bf, isOutput=False)
    embT = nc.declare_dram_parameter("embT", [NCH, 128, KB, CHUNK], f8, isOutput=False)
    out = nc.declare_dram_parameter("out", [S, VP], f32, isOutput=True)

    # partition-major AllGather buffers: per-rank (128, jobs, S)
    ehl = [
        nc.dram_tensor(f"eh_local{p}", [128, AGP[p], S], f8)
        for p in range(len(AGP))
    ]
    eha = [
        nc.dram_tensor(
            f"eh_all{p}", [NCORES * 128, AGP[p], S], f8, addr_space="Shared"
        )
        for p in range(len(AGP))
    ]
    zl = [nc.dram_tensor(f"zl{sh}", [128, E], f32) for sh in range(2)]
    za = [
        nc.dram_tensor(f"za{sh}", [128, E], f32, addr_space="Shared")
        for sh in range(2)
    ]
    xsp = nc.dram_tensor("xspill", [NCH, 2, 128, E, CHUNK], bf)

    rg = [list(range(NCORES))]

    with tile.TileContext(nc) as tc:
        with (
            tc.tile_pool(name="et", bufs=3) as etp,          # 10KB slots
            tc.tile_pool(name="stream", bufs=4) as strm,     # 10KB slots
            tc.tile_pool(name="xs", bufs=3) as xsp_pool,     # 10KB slots
            tc.tile_pool(name="ehsh", bufs=1) as ehp,        # 10 x 5KB
            tc.tile_pool(name="stage", bufs=3) as stgp,      # 4.3KB slots
            tc.tile_pool(name="acc", bufs=3) as accp,
            tc.tile_pool(name="ot", bufs=2) as otp,
            tc.tile_pool(name="persist", bufs=1) as per,
            tc.tile_pool(name="psmall", bufs=4, space="PSUM") as psS,
            tc.tile_pool(name="psbig", bufs=4, space="PSUM") as psC,
        ):
            epsn = per.tile([128, 1], f32, tag="epsn")
            nc.vector.memset(epsn, EPS_NORM)
            epsl = per.tile([128, 1], f32, tag="epsl")
            nc.vector.memset(epsl, EPS_LOG)

            # raw h^T (bf16) — expert matmuls use it un-normalized
            hTr = strm.tile([128, KB, S], bf, tag="stream")
            nc.sync.dma_start(out=hTr, in_=hT[:])

            # ---- per-token RMS factors r_s = 1/sqrt(mean(h^2)+eps) ----
            # mean(h^2) = var + mean^2 via bn_stats (no large temps)
            NSG = H // nc.vector.BN_STATS_FMAX
            r = []
            for sh in range(2):
                ht = etp.tile([128, H], f32, tag="et")
                nc.sync.dma_start(out=ht, in_=h32[sh])
                stats = per.tile(
                    [128, NSG, nc.vector.BN_STATS_DIM], f32, tag=f"st{sh}"
                )
                for sg in range(NSG):
                    nc.vector.bn_stats(
                        out=stats[:, sg, :],
                        in_=ht[
                            :,
                            sg * nc.vector.BN_STATS_FMAX : (sg + 1)
                            * nc.vector.BN_STATS_FMAX,
                        ],
                    )
                mv = per.tile([128, nc.vector.BN_AGGR_DIM], f32, tag=f"mv{sh}")
                nc.vector.bn_aggr(out=mv, in_=stats)
                msq = per.tile([128, 1], f32, tag=f"msq{sh}")
                nc.vector.scalar_tensor_tensor(
                    out=msq, in0=mv[:, 0:1], scalar=mv[:, 0:1], in1=mv[:, 1:2],
                    op0=mybir.AluOpType.mult, op1=mybir.AluOpType.add,
                )
                rsd = per.tile([128, 1], f32, tag=f"rsd{sh}")
                nc.scalar.activation(
                    out=rsd, in_=msq, func=mybir.ActivationFunctionType.Sqrt,
                    bias=epsn[:, 0:1],
                )
                rt = per.tile([128, 1], f32, tag=f"r{sh}")
                nc.vector.reciprocal(rt, rsd)
                r.append(rt)

            # broadcast r over partitions via PE (no DRAM roundtrip):
            # rT = r^T (1,256) by PE transpose; rbc16 = (EH_SCALE*ones) x rT
            ident = per.tile([128, 128], f32, tag="ident")
            make_identity(nc, ident)
            rT = psS.tile([1, S], f32, tag="ps_small")
            nc.tensor.transpose(rT[:, 0:128], r[0], ident)
            nc.tensor.transpose(rT[:, 128:256], r[1], ident)
            rTs = per.tile([1, S], f32, tag="rTs")
            nc.vector.tensor_copy(rTs, rT)
            sc1 = per.tile([1, 128], f32, tag="sc1")
            nc.vector.memset(sc1, EH_SCALE)
            rbps = psS.tile([128, S], f32, tag="ps_small")
            nc.tensor.matmul(rbps, sc1, rTs, start=True, stop=True)
            rbc16 = per.tile([128, S], f32, tag="rbc16")
            nc.vector.tensor_copy(rbc16, rbps)

            # ---- expert transform shard: 25 (e, dblk) jobs; r applied in copy
            elbufs = [
                per.tile([128, AGP[p], S], f8, tag=f"elbuf{p}", name=f"elbuf{p}")
                for p in range(len(AGP))
            ]
            for j in range(JPC):
                wjt = strm.tile([128, KB, 128], bf, tag="stream")
                nc.sync.dma_start(out=wjt, in_=wj[j])
                bps = psS.tile([128, S], f32, tag="ps_small")
                for k in range(KB):
                    nc.tensor.matmul(
                        bps, wjt[:, k, :], hTr[:, k, :],
                        start=(k == 0), stop=(k == KB - 1),
                    )
                for p in range(len(AGP)):
                    if AGOFF[p] <= j < AGOFF[p] + AGP[p]:
                        nc.vector.tensor_mul(
                            elbufs[p][:, j - AGOFF[p], :], bps, rbc16
                        )
                        if j == AGOFF[p] + AGP[p] - 1:
                            nc.sync.dma_start(out=ehl[p][:], in_=elbufs[p])
                            nc.gpsimd.collective_compute(
                                "AllGather", mybir.AluOpType.bypass,
                                replica_groups=rg,
                                ins=[ehl[p][:]], outs=[eha[p][:]],
                            )

            # ---- gate softmax g (no max shift; logits are O(1)) ----
            gw3 = per.tile([128, KB, E], bf, tag="gw3")
            nc.sync.dma_start(out=gw3, in_=gw[:].rearrange("k p e -> p k e"))
            g = []
            for sh in range(2):
                gps = psS.tile([128, E], f32, tag="ps_small")
                for k in range(KB):
                    nc.tensor.matmul(
                        gps,
                        hTr[:, k, sh * 128 : (sh + 1) * 128],
                        gw3[:, k, :],
                        start=(k == 0),
                        stop=(k == KB - 1),
                    )
                ge = per.tile([128, E], f32, tag=f"ge{sh}")
                gsum = per.tile([128, 1], f32, tag=f"gsum{sh}")
                nc.scalar.activation(
                    out=ge, in_=gps, func=mybir.ActivationFunctionType.Exp,
                    scale=r[sh][:, 0:1], accum_out=gsum[:, 0:1],
                )
                grc = per.tile([128, 1], f32, tag=f"grc{sh}")
                nc.vector.reciprocal(grc, gsum)
                gt = per.tile([128, E], f32, tag=f"g{sh}")
                nc.vector.tensor_scalar_mul(gt, ge, grc[:, 0:1])
                g.append(gt)

            # ---- gather expert_hidden^T into per-expert SBUF tiles ----
            # big-descriptor stage loads (p-major), then engine copies
            ehsh = [
                ehp.tile([128, KB, S], f8, tag=f"ehsh{e}", name=f"ehsh{e}")
                for e in range(E)
            ]
            ncopy = 0
            for p in range(len(AGP)):
                for c in range(NCORES):
                    stp = stgp.tile(
                        [128, AGP[p], S], f8, tag="stage", name=f"st{p}_{c}"
                    )
                    nc.sync.dma_start(
                        out=stp, in_=eha[p][c * 128 : (c + 1) * 128, :, :]
                    )
                    for jj in range(AGP[p]):
                        e, k = divmod(_job_of(c, AGOFF[p] + jj), KB)
                        eng = nc.vector if ncopy % 2 == 0 else nc.gpsimd
                        eng.tensor_copy(
                            out=ehsh[e][:, k, :].bitcast(u8),
                            in_=stp[:, jj, :].bitcast(u8),
                        )
                        ncopy += 1

            # ---- main: per s-half ----
            for sh in range(2):
                zacc = per.tile([128, E], f32, tag=f"zacc{sh}")
                nc.vector.memset(zacc, 0.0)

                # pass 1: logits -> exp -> spill; accumulate Z row-sums
                kept = {}
                for ci, (v0, vn) in enumerate(CHUNKS):
                    et3 = etp.tile([128, KB, CHUNK], f8, tag="et")
                    nc.sync.dma_start(out=et3, in_=embT[ci])
                    xs3 = xsp_pool.tile([128, E, CHUNK], bf, tag="xs")
                    if vn < CHUNK:
                        nc.vector.memset(xs3[:, :, vn:], 0.0)
                    for e in range(E):
                        cps = psC.tile([128, CHUNK], f32, tag="psC")
                        for k2 in range(KB // 2):
                            nc.tensor.matmul(
                                cps[:, :vn],
                                ehsh[e][:, 2 * k2 : 2 * k2 + 2,
                                        sh * 128 : (sh + 1) * 128],
                                et3[:, 2 * k2 : 2 * k2 + 2, :vn],
                                start=(k2 == 0),
                                stop=(k2 == KB // 2 - 1),
                                perf_mode=mybir.MatmulPerfMode.DoubleRow,
                            )
                        zc = per.tile([128, 1], f32, tag="zc", bufs=8)
                        nc.scalar.activation(
                            out=xs3[:, e, :vn], in_=cps[:, :vn],
                            func=mybir.ActivationFunctionType.Exp,
                            scale=INV_SCALE,
                            accum_out=zc[:, 0:1],
                        )
                        nc.gpsimd.tensor_add(
                            zacc[:, e : e + 1], zacc[:, e : e + 1], zc
                        )
                    if sh == 1 and ci >= NCH - 2:
                        kept[ci] = xs3
                    else:
                        nc.sync.dma_start(out=xsp[ci, sh], in_=xs3)

                # Z AllReduce + pad correction + R = g / Z  (bf16 for mixing)
                nc.sync.dma_start(out=zl[sh][:], in_=zacc)
                nc.gpsimd.collective_compute(
                    "AllReduce", mybir.AluOpType.add, replica_groups=rg,
                    ins=[zl[sh][:]], outs=[za[sh][:]],
                )
                zs = per.tile([128, E], f32, tag=f"zs{sh}")
                nc.sync.dma_start(out=zs, in_=za[sh][:])
                nc.vector.tensor_scalar_add(zs, zs, float(-NPAD))
                zrc = per.tile([128, E], f32, tag=f"zrc{sh}")
                nc.vector.reciprocal(zrc, zs)
                Rt = per.tile([128, E], f32, tag=f"R{sh}")
                nc.vector.tensor_mul(Rt, g[sh], zrc)

                # pass 2: mixed = sum_e R_e * X_e ; out = ln(mixed + eps)
                for ci, (v0, vn) in enumerate(CHUNKS):
                    if ci in kept:
                        xt3 = kept[ci]
                    else:
                        xt3 = strm.tile(
                            [128, E, CHUNK], bf, tag="stream",
                            name=f"xt3_{sh}_{ci}",
                        )
                        nc.sync.dma_start(out=xt3, in_=xsp[ci, sh])
                    # hybrid mix: DVE fused chain for e0..5; ACT scales
                    # e6..9 in place; GpSimd adds those; DVE merges once
                    accd = accp.tile([128, CHUNK], bf, tag="accd")
                    nc.vector.tensor_scalar_mul(
                        accd[:, :vn], xt3[:, 0, :vn], Rt[:, 0:1]
                    )
                    for e in range(1, 5):
                        nc.vector.scalar_tensor_tensor(
                            out=accd[:, :vn],
                            in0=xt3[:, e, :vn],
                            scalar=Rt[:, e : e + 1],
                            in1=accd[:, :vn],
                            op0=mybir.AluOpType.mult,
                            op1=mybir.AluOpType.add,
                        )
                    x = lambda e: xt3[:, e, :vn]
                    for e in range(5, E):
                        nc.scalar.activation(
                            out=x(e), in_=x(e),
                            func=mybir.ActivationFunctionType.Copy,
                            scale=Rt[:, e : e + 1],
                        )
                    nc.gpsimd.tensor_add(x(5), x(5), x(6))
                    nc.gpsimd.tensor_add(x(7), x(7), x(8))
                    nc.gpsimd.tensor_add(x(5), x(5), x(9))
                    nc.gpsimd.tensor_add(x(5), x(5), x(7))
                    nc.vector.tensor_add(accd[:, :vn], accd[:, :vn], x(5))
                    ot = otp.tile([128, CHUNK], f32, tag="ot")
                    nc.scalar.activation(
                        out=ot[:, :vn], in_=accd[:, :vn],
                        func=mybir.ActivationFunctionType.Ln,
                        bias=epsl[:, 0:1],
                    )
                    nc.sync.dma_start(
                        out=out[sh * 128 : (sh + 1) * 128, v0 : v0 + vn],
                        in_=ot[:, :vn],
                    )

    nc.compile()
    _nc_cache = nc
    return nc


def prepare_in_maps(inputs):
    h = np.asarray(inputs["hidden_states"], np.float32).reshape(S, H)
    emb = np.asarray(inputs["embedding_matrix"], np.float32)
    ns = np.asarray(inputs["norm_scale"], np.float32)
    W = np.asarray(inputs["expert_weights"], np.float32)
    G = np.asarray(inputs["gate_weight"], np.float32)

    h32 = np.ascontiguousarray(h.reshape(2, 128, H))
    # hT[p, k, s] = h[s, k*128+p]
    hTb = np.ascontiguousarray(h.reshape(S, KB, 128).transpose(2, 1, 0)).astype(BF16)
    gwb = np.ascontiguousarray((G * ns[:, None]).reshape(KB, 128, E)).astype(BF16)

    Wn = W * ns[None, :, None]
    # wjobs_all[j = e*KB + dblk, p, k, d] = Wn[e, k*128+p, dblk*128+d]
    Wr = Wn.reshape(E, KB, 128, KB, 128)
    wjobs_all = np.ascontiguousarray(
        Wr.transpose(0, 3, 2, 1, 4).reshape(NJOBS, 128, KB, 128)
    ).astype(BF16)

    VPAD = NCH * CHUNK  # 6656 (layout padding only; compute uses VP)
    embp = np.zeros((NCORES * VP + (VPAD - VP), H), np.float32)
    embp[:V] = emb

    job_order = [
        [(_job_of(c, slot)) for slot in range(JPC)] for c in range(NCORES)
    ]

    in_maps = []
    for c in range(NCORES):
        esl = embp[c * VP : c * VP + VPAD]  # (VPAD, H) with layout pad tail
        # embT_c[ci, p, k, v] = esl[ci*CHUNK+v, k*128+p] * EMB_SCALE
        embT_c = (
            np.ascontiguousarray(
                esl.reshape(NCH, CHUNK, KB, 128).transpose(0, 3, 2, 1)
            )
            * EMB_SCALE
        ).astype(FP8)
        in_maps.append(
            {
                "h32": h32,
                "hT": hTb,
                "gw": gwb,
                "wjobs": np.ascontiguousarray(wjobs_all[job_order[c]]),
                "embT": embT_c,
            }
        )
    return in_maps


def assemble_output(results):
    full = np.concatenate([results[c]["out"] for c in range(NCORES)], axis=1)
    return np.ascontiguousarray(full[:, :V].reshape(1, S, V).astype(np.float32))


def kernel(**inputs):
    nc = build_kernel()
    in_maps = prepare_in_maps(inputs)
    res = run_bass_kernel_spmd(nc, in_maps, list(range(NCORES)))
    return assemble_output(res.results)


# revision 22
# speedup vs baseline: 1.0492x; 1.0492x over previous
"""Mixture-of-Softmaxes with shared embedding — 8-core Trainium2 Bass kernel.

Strategy (tensor-parallel on the vocab output head, per the sharding hint):
  - Vocab dim V is sharded across the 8 cores (Vp = 6283 rows each, zero-padded
    from 50257 to 50264; the 7 pad rows contribute exactly exp(0)=1 to each
    softmax denominator and are corrected by a constant subtraction).
  - The expert transforms (10 experts x 2560x2560) are sharded as 200
    (expert, d-block) jobs, 25 per core, followed by an AllGather of the
    fp8 expert_hidden^T (6.5 MB, split 17/8 so the first piece overlaps the
    tail of the expert matmuls).
  - The big vocab matmul runs in fp8 (e4m3) with DoubleRow perf mode
    (K=256 per instruction): embedding is scaled x64 on the host,
    expert_hidden x16 on chip, undone by the fused exp scale (1/1024).
  - Softmax over the full vocab needs a cross-core reduction: since the
    logits are O(1), exp() cannot overflow in fp32, so the max-shift is
    unnecessary and the reduction collapses to one AllReduce-ADD of the
    (128,10) sum-of-exp stats per token-half.
  - RMSNorm is folded in linearly: norm_scale is folded into the expert and
    gate weights on the host; the per-token 1/rms factor is computed on-chip
    in fp32 and applied during the PSUM->SBUF copy of expert_hidden
    (free-dim broadcast), so the expert matmuls never wait on it.
  - All DRAM layouts are partition-major so DMA descriptors move >=5KB
    contiguous runs per partition (descriptor-rate, not bytes, was the
    limiting factor for the spill/reload streams).

kernel(**inputs) takes the full unsharded inputs and returns the full
(1, 256, 50257) float32 logits.
"""
import os
import sys

for _p in ("/opt/trn_rl_repo",):
    if _p not in sys.path:
        sys.path.append(_p)

import numpy as np
import ml_dtypes

import concourse.bass as bass
import concourse.bacc as bacc
import concourse.mybir as mybir
import concourse.tile as tile
from concourse.bass_utils import run_bass_kernel_spmd
from concourse.masks import make_identity

BF16 = ml_dtypes.bfloat16
FP8 = ml_dtypes.float8_e4m3

NCORES = 8
S = 256          # tokens (B*S)
H = 2560         # hidden
E = 10           # experts
V = 50257        # vocab
KB = H // 128    # 20 k-blocks
NJOBS = E * KB   # 200 (expert, d-block) jobs
JPC = NJOBS // NCORES  # 25 jobs per core
AGP = (9, 8, 8)  # jobs per core in each AllGather piece
AGOFF = (0, 9, 17)  # slot offsets of the pieces
VP = 6283        # per-core vocab slice (8*6283 = 50264)
NPAD = NCORES * VP - V  # 7 zero-pad vocab rows (on the last core)
CHUNK = 512
NCH = (VP + CHUNK - 1) // CHUNK  # 13
CHUNKS = [(i * CHUNK, min(CHUNK, VP - i * CHUNK)) for i in range(NCH)]
EPS_NORM = 1e-05
EPS_LOG = 1e-10
EMB_SCALE = 64.0
EH_SCALE = 16.0
INV_SCALE = 1.0 / (EMB_SCALE * EH_SCALE)
GP_E = (7, 8, 9)   # experts mixed on GpSimd in pass 2 (rest on DVE)

_nc_cache = None


def _job_of(c, slot):
    """Global job id (= e*KB + k) held by core c at slot (0..JPC-1).

    AG piece p carries a rank-major contiguous range of global jobs, so
    low experts are complete as soon as the early pieces land.
    """
    for p, (n, off) in enumerate(zip(AGP, AGOFF)):
        if slot < off + n:
            base = NCORES * sum(AGP[:p])
            return base + c * n + (slot - off)
    raise ValueError(slot)


def build_kernel():
    global _nc_cache
    if _nc_cache is not None:
        return _nc_cache
    f32 = mybir.dt.float32
    bf = mybir.dt.bfloat16
    f8 = mybir.dt.float8e4
    u8 = mybir.dt.uint8
    nc = bacc.Bacc("TRN2", target_bir_lowering=False, debug=False, num_devices=NCORES)

    h32 = nc.declare_dram_parameter("h32", [2, 128, H], f32, isOutput=False)
    hT = nc.declare_dram_parameter("hT", [128, KB, S], bf, isOutput=False)
    gw = nc.declare_dram_parameter("gw", [KB, 128, E], bf, isOutput=False)
    wj = nc.declare_dram_parameter("wjobs", [JPC, 128, KB, 128], bf, isOutput=False)
    embT = nc.declare_dram_parameter("embT", [NCH, 128, KB, CHUNK], f8, isOutput=False)
    out = nc.declare_dram_parameter("out", [S, VP], f32, isOutput=True)

    # partition-major AllGather buffers: per-rank (128, jobs, S)
    ehl = [
        nc.dram_tensor(f"eh_local{p}", [128, AGP[p], S], f8)
        for p in range(len(AGP))
    ]
    eha = [
        nc.dram_tensor(
            f"eh_all{p}", [NCORES * 128, AGP[p], S], f8, addr_space="Shared"
        )
        for p in range(len(AGP))
    ]
    zl = [nc.dram_tensor(f"zl{sh}", [128, E], f32) for sh in range(2)]
    za = [
        nc.dram_tensor(f"za{sh}", [128, E], f32, addr_space="Shared")
        for sh in range(2)
    ]
    xsp = nc.dram_tensor("xspill", [NCH, 2, 128, E, CHUNK], bf)

    rg = [list(range(NCORES))]

    with tile.TileContext(nc) as tc:
        with (
            tc.tile_pool(name="et", bufs=3) as etp,          # 10KB slots
            tc.tile_pool(name="stream", bufs=4) as strm,     # 10KB slots
            tc.tile_pool(name="xs", bufs=3) as xsp_pool,     # 10KB slots
            tc.tile_pool(name="ehsh", bufs=1) as ehp,        # 10 x 5KB
            tc.tile_pool(name="stage", bufs=3) as stgp,      # 4.3KB slots
            tc.tile_pool(name="acc", bufs=3) as accp,
            tc.tile_pool(name="ot", bufs=2) as otp,
            tc.tile_pool(name="persist", bufs=1) as per,
            tc.tile_pool(name="psmall", bufs=4, space="PSUM") as psS,
            tc.tile_pool(name="psbig", bufs=4, space="PSUM") as psC,
        ):
            epsn = per.tile([128, 1], f32, tag="epsn")
            nc.vector.memset(epsn, EPS_NORM)
            epsl = per.tile([128, 1], f32, tag="epsl")
            nc.vector.memset(epsl, EPS_LOG)

            # raw h^T (bf16) — expert matmuls use it un-normalized
            hTr = strm.tile([128, KB, S], bf, tag="stream")
            nc.sync.dma_start(out=hTr, in_=hT[:])

            # ---- per-token RMS factors r_s = 1/sqrt(mean(h^2)+eps) ----
            # mean(h^2) = var + mean^2 via bn_stats (no large temps)
            NSG = H // nc.vector.BN_STATS_FMAX
            r = []
            for sh in range(2):
                ht = etp.tile([128, H], f32, tag="et")
                nc.sync.dma_start(out=ht, in_=h32[sh])
                stats = per.tile(
                    [128, NSG, nc.vector.BN_STATS_DIM], f32, tag=f"st{sh}"
                )
                for sg in range(NSG):
                    nc.vector.bn_stats(
                        out=stats[:, sg, :],
                        in_=ht[
                            :,
                            sg * nc.vector.BN_STATS_FMAX : (sg + 1)
                            * nc.vector.BN_STATS_FMAX,
                        ],
                    )
                mv = per.tile([128, nc.vector.BN_AGGR_DIM], f32, tag=f"mv{sh}")
                nc.vector.bn_aggr(out=mv, in_=stats)
                msq = per.tile([128, 1], f32, tag=f"msq{sh}")
                nc.vector.scalar_tensor_tensor(
                    out=msq, in0=mv[:, 0:1], scalar=mv[:, 0:1], in1=mv[:, 1:2],
                    op0=mybir.AluOpType.mult, op1=mybir.AluOpType.add,
                )
                rsd = per.tile([128, 1], f32, tag=f"rsd{sh}")
                nc.scalar.activation(
                    out=rsd, in_=msq, func=mybir.ActivationFunctionType.Sqrt,
                    bias=epsn[:, 0:1],
                )
                rt = per.tile([128, 1], f32, tag=f"r{sh}")
                nc.vector.reciprocal(rt, rsd)
                r.append(rt)

            # broadcast r over partitions via PE (no DRAM roundtrip):
            # rT = r^T (1,256) by PE transpose; rbc16 = (EH_SCALE*ones) x rT
            ident = per.tile([128, 128], f32, tag="ident")
            make_identity(nc, ident)
            rT = psS.tile([1, S], f32, tag="ps_small")
            nc.tensor.transpose(rT[:, 0:128], r[0], ident)
            nc.tensor.transpose(rT[:, 128:256], r[1], ident)
            rTs = per.tile([1, S], f32, tag="rTs")
            nc.vector.tensor_copy(rTs, rT)
            sc1 = per.tile([1, 128], f32, tag="sc1")
            nc.vector.memset(sc1, EH_SCALE)
            rbps = psS.tile([128, S], f32, tag="ps_small")
            nc.tensor.matmul(rbps, sc1, rTs, start=True, stop=True)
            rbc16 = per.tile([128, S], f32, tag="rbc16")
            nc.vector.tensor_copy(rbc16, rbps)

            # ---- expert transform shard: 25 (e, dblk) jobs; r applied in copy
            elbufs = [
                per.tile([128, AGP[p], S], f8, tag=f"elbuf{p}", name=f"elbuf{p}")
                for p in range(len(AGP))
            ]
            for j in range(JPC):
                wjt = strm.tile([128, KB, 128], bf, tag="stream")
                nc.sync.dma_start(out=wjt, in_=wj[j])
                bps = psS.tile([128, S], f32, tag="ps_small")
                for k in range(KB):
                    nc.tensor.matmul(
                        bps, wjt[:, k, :], hTr[:, k, :],
                        start=(k == 0), stop=(k == KB - 1),
                    )
                for p in range(len(AGP)):
                    if AGOFF[p] <= j < AGOFF[p] + AGP[p]:
                        nc.vector.tensor_mul(
                            elbufs[p][:, j - AGOFF[p], :], bps, rbc16
                        )
                        if j == AGOFF[p] + AGP[p] - 1:
                            nc.sync.dma_start(out=ehl[p][:], in_=elbufs[p])
                            nc.gpsimd.collective_compute(
                                "AllGather", mybir.AluOpType.bypass,
                                replica_groups=rg,
                                ins=[ehl[p][:]], outs=[eha[p][:]],
                            )

            # ---- gate softmax g (no max shift; logits are O(1)) ----
            gw3 = per.tile([128, KB, E], bf, tag="gw3")
            nc.sync.dma_start(out=gw3, in_=gw[:].rearrange("k p e -> p k e"))
            g = []
            for sh in range(2):
                gps = psS.tile([128, E], f32, tag="ps_small")
                for k in range(KB):
                    nc.tensor.matmul(
                        gps,
                        hTr[:, k, sh * 128 : (sh + 1) * 128],
                        gw3[:, k, :],
                        start=(k == 0),
                        stop=(k == KB - 1),
                    )
                ge = per.tile([128, E], f32, tag=f"ge{sh}")
                gsum = per.tile([128, 1], f32, tag=f"gsum{sh}")
                nc.scalar.activation(
                    out=ge, in_=gps, func=mybir.ActivationFunctionType.Exp,
                    scale=r[sh][:, 0:1], accum_out=gsum[:, 0:1],
                )
                grc = per.tile([128, 1], f32, tag=f"grc{sh}")
                nc.vector.reciprocal(grc, gsum)
                gt = per.tile([128, E], f32, tag=f"g{sh}")
                nc.vector.tensor_scalar_mul(gt, ge, grc[:, 0:1])
                g.append(gt)

            # ---- gather expert_hidden^T into per-expert SBUF tiles ----
            # big-descriptor stage loads (p-major), then engine copies
            ehsh = [
                ehp.tile([128, KB, S], f8, tag=f"ehsh{e}", name=f"ehsh{e}")
                for e in range(E)
            ]
            ncopy = 0
            for p in range(len(AGP)):
                for c in range(NCORES):
                    stp = stgp.tile(
                        [128, AGP[p], S], f8, tag="stage", name=f"st{p}_{c}"
                    )
                    nc.sync.dma_start(
                        out=stp, in_=eha[p][c * 128 : (c + 1) * 128, :, :]
                    )
                    for jj in range(AGP[p]):
                        e, k = divmod(_job_of(c, AGOFF[p] + jj), KB)
                        eng = nc.vector if ncopy % 2 == 0 else nc.gpsimd
                        eng.tensor_copy(
                            out=ehsh[e][:, k, :].bitcast(u8),
                            in_=stp[:, jj, :].bitcast(u8),
                        )
                        ncopy += 1

            # ---- main: per s-half ----
            for sh in range(2):
                zacc = per.tile([128, E], f32, tag=f"zacc{sh}")
                nc.vector.memset(zacc, 0.0)

                # pass 1: logits -> exp -> spill; accumulate Z row-sums
                kept = {}
                for ci, (v0, vn) in enumerate(CHUNKS):
                    et3 = etp.tile([128, KB, CHUNK], f8, tag="et")
                    nc.sync.dma_start(out=et3, in_=embT[ci])
                    xs3 = xsp_pool.tile([128, E, CHUNK], bf, tag="xs")
                    if vn < CHUNK:
                        nc.vector.memset(xs3[:, :, vn:], 0.0)
                    for e in range(E):
                        cps = psC.tile([128, CHUNK], f32, tag="psC")
                        for k2 in range(KB // 2):
                            nc.tensor.matmul(
                                cps[:, :vn],
                                ehsh[e][:, 2 * k2 : 2 * k2 + 2,
                                        sh * 128 : (sh + 1) * 128],
                                et3[:, 2 * k2 : 2 * k2 + 2, :vn],
                                start=(k2 == 0),
                                stop=(k2 == KB // 2 - 1),
                                perf_mode=mybir.MatmulPerfMode.DoubleRow,
                            )
                        zc = per.tile([128, 1], f32, tag="zc", bufs=8)
                        nc.scalar.activation(
                            out=xs3[:, e, :vn], in_=cps[:, :vn],
                            func=mybir.ActivationFunctionType.Exp,
                            scale=INV_SCALE,
                            accum_out=zc[:, 0:1],
                        )
                        nc.gpsimd.tensor_add(
                            zacc[:, e : e + 1], zacc[:, e : e + 1], zc
                        )
                    if sh == 1 and ci >= NCH - 2:
                        kept[ci] = xs3
                    else:
                        nc.sync.dma_start(out=xsp[ci, sh], in_=xs3)

                # Z AllReduce + pad correction + R = g / Z  (bf16 for mixing)
                nc.sync.dma_start(out=zl[sh][:], in_=zacc)
                nc.gpsimd.collective_compute(
                    "AllReduce", mybir.AluOpType.add, replica_groups=rg,
                    ins=[zl[sh][:]], outs=[za[sh][:]],
                )
                zs = per.tile([128, E], f32, tag=f"zs{sh}")
                nc.sync.dma_start(out=zs, in_=za[sh][:])
                nc.vector.tensor_scalar_add(zs, zs, float(-NPAD))
                zrc = per.tile([128, E], f32, tag=f"zrc{sh}")
                nc.vector.reciprocal(zrc, zs)
                Rt = per.tile([128, E], f32, tag=f"R{sh}")
                nc.vector.tensor_mul(Rt, g[sh], zrc)

                # pass 2: mixed = sum_e R_e * X_e ; out = ln(mixed + eps)
                for ci, (v0, vn) in enumerate(CHUNKS):
                    if ci in kept:
                        xt3 = kept[ci]
                    else:
                        xt3 = strm.tile(
                            [128, E, CHUNK], bf, tag="stream",
                            name=f"xt3_{sh}_{ci}",
                        )
                        nc.sync.dma_start(out=xt3, in_=xsp[ci, sh])
                    # hybrid mix: DVE fused chain for e0..5; ACT scales
                    # e6..9 in place; GpSimd adds those; DVE merges once
                    accd = accp.tile([128, CHUNK], bf, tag="accd")
                    nc.vector.tensor_scalar_mul(
                        accd[:, :vn], xt3[:, 0, :vn], Rt[:, 0:1]
                    )
                    for e in range(1, E):
                        nc.vector.scalar_tensor_tensor(
                            out=accd[:, :vn],
                            in0=xt3[:, e, :vn],
                            scalar=Rt[:, e : e + 1],
                            in1=accd[:, :vn],
                            op0=mybir.AluOpType.mult,
                            op1=mybir.AluOpType.add,
                        )
                    ot = otp.tile([128, CHUNK], f32, tag="ot")
                    nc.scalar.activation(
                        out=ot[:, :vn], in_=accd[:, :vn],
                        func=mybir.ActivationFunctionType.Ln,
                        bias=epsl[:, 0:1],
                    )
                    nc.sync.dma_start(
                        out=out[sh * 128 : (sh + 1) * 128, v0 : v0 + vn],
                        in_=ot[:, :vn],
                    )

    nc.compile()
    _nc_cache = nc
    return nc


def prepare_in_maps(inputs):
    h = np.asarray(inputs["hidden_states"], np.float32).reshape(S, H)
    emb = np.asarray(inputs["embedding_matrix"], np.float32)
    ns = np.asarray(inputs["norm_scale"], np.float32)
    W = np.asarray(inputs["expert_weights"], np.float32)
    G = np.asarray(inputs["gate_weight"], np.float32)

    h32 = np.ascontiguousarray(h.reshape(2, 128, H))
    # hT[p, k, s] = h[s, k*128+p]
    hTb = np.ascontiguousarray(h.reshape(S, KB, 128).transpose(2, 1, 0)).astype(BF16)
    gwb = np.ascontiguousarray((G * ns[:, None]).reshape(KB, 128, E)).astype(BF16)

    Wn = W * ns[None, :, None]
    # wjobs_all[j = e*KB + dblk, p, k, d] = Wn[e, k*128+p, dblk*128+d]
    Wr = Wn.reshape(E, KB, 128, KB, 128)
    wjobs_all = np.ascontiguousarray(
        Wr.transpose(0, 3, 2, 1, 4).reshape(NJOBS, 128, KB, 128)
    ).astype(BF16)

    VPAD = NCH * CHUNK  # 6656 (layout padding only; compute uses VP)
    embp = np.zeros((NCORES * VP + (VPAD - VP), H), np.float32)
    embp[:V] = emb

    job_order = [
        [(_job_of(c, slot)) for slot in range(JPC)] for c in range(NCORES)
    ]

    in_maps = []
    for c in range(NCORES):
        esl = embp[c * VP : c * VP + VPAD]  # (VPAD, H) with layout pad tail
        # embT_c[ci, p, k, v] = esl[ci*CHUNK+v, k*128+p] * EMB_SCALE
        embT_c = (
            np.ascontiguousarray(
                esl.reshape(NCH, CHUNK, KB, 128).transpose(0, 3, 2, 1)
            )
            * EMB_SCALE
        ).astype(FP8)
        in_maps.append(
            {
                "h32": h32,
                "hT": hTb,
                "gw": gwb,
                "wjobs": np.ascontiguousarray(wjobs_all[job_order[c]]),
                "embT": embT_c,
            }
        )
    return in_maps


def assemble_output(results):
    full = np.concatenate([results[c]["out"] for c in range(NCORES)], axis=1)
    return np.ascontiguousarray(full[:, :V].reshape(1, S, V).astype(np.float32))


def kernel(**inputs):
    nc = build_kernel()
    in_maps = prepare_in_maps(inputs)
    res = run_bass_kernel_spmd(nc, in_maps, list(range(NCORES)))
    return assemble_output(res.results)
